# revision 1
# baseline (speedup 1.0000x reference)
"""2D-DCT (DCT-II, orthonormal) spatial transform on Trainium2, 8 NeuronCores.

Full input x [16,256,128,128] f32 -> out[b,c,k,v] = sum_hw Wy[k,h] Wx[v,w] x[b,c,h,w]
with Wy = Wx = 128-point orthonormal DCT-II matrix W.

Strategy (data-parallel, batch*channel sharded 4096 -> 512 images/core):
per image X: out = W @ X @ W.T, computed with two PE matmuls and zero
explicit transposes -- matmul(out, lhsT, rhs) = lhsT.T @ rhs transposes the
stationary operand for free:
  mm1 (fp32, exact):  lhsT=X_i,  rhs=W.T            -> Z^T = (W@X)^T   (PSUM)
  mm2 (float32r):     lhsT=Z^T_i, rhs=[W.T|W.T]     -> [out|out]       (PSUM)
float32r is the TF32-like (8e11m) PE mode: 1 cycle/row when the moving free
dim is >=256 (vs 4 cycles/row for fp32). Only Z^T and W round to tf32;
error ~1.5e-4 scale-relative. PSUM->SBUF copies are batched (4 images for
stage 1 on ACT, 2 for stage 2 on DVE) and PSUM banks are packed so both
vector engines stay under the HBM roofline (~366 ns/img).
"""

import sys

for _p in ("/opt/trn_rl_repo", "/root/.axon_site/_ro/trn_rl_repo"):
    if _p not in sys.path:
        sys.path.insert(0, _p)

import numpy as np

N_CORES = 8
B, C, H, W = 16, 256, 128, 128
PER_CORE = B * C // N_CORES  # 512 images per core


def _dct_matrix(n: int) -> np.ndarray:
    v = np.arange(n, dtype=np.float64)[:, None]
    j = np.arange(n, dtype=np.float64)[None, :]
    f = np.cos(np.pi * (0.5 + j) * v / n) / np.sqrt(n)
    f *= np.where(v != 0, np.sqrt(2.0), 1.0)
    return f.astype(np.float32)


def _build_program(n_img: int, group: int = 8, xg_bufs: int = 4, og_bufs: int = 4,
                   p1_bufs: int = 3, p2_bufs: int = 4, zt_bufs: int = 3):
    import concourse.bacc as bacc_mod
    import concourse.mybir as mybir
    from concourse.tile import TileContext

    F32 = mybir.dt.float32
    F32R = mybir.dt.float32r

    nc = bacc_mod.Bacc()
    x = nc.declare_dram_parameter("x", [n_img, 128, 128], F32, isOutput=False)
    wt_p = nc.declare_dram_parameter("wt", [128, 128], F32, isOutput=False)
    wr2_p = nc.declare_dram_parameter("wr2", [128, 256], F32, isOutput=False)
    out = nc.declare_dram_parameter("out", [n_img, 128, 128], F32, isOutput=True)

    with TileContext(nc) as tc:
        with tc.tile_pool(name="consts", bufs=1) as cpool, \
             tc.tile_pool(name="xin", bufs=xg_bufs) as xpool, \
             tc.tile_pool(name="mid", bufs=zt_bufs) as zpool, \
             tc.tile_pool(name="oput", bufs=og_bufs) as opool, \
             tc.tile_pool(name="ps", bufs=1, space="PSUM") as pspool:
            wt = cpool.tile([128, 128], F32)
            nc.sync.dma_start(out=wt, in_=wt_p[:])
            wr2 = cpool.tile([128, 256], F32R)
            nc.gpsimd.dma_start(out=wr2, in_=wr2_p[:])

            # PE warm-up dummies: ensure no later (self-loading) matmul needs
            # more than one sync wait -- the S3_LW struct can carry only one.
            pdum = pspool.tile([128, 128], F32, tag="pdum", bufs=1)
            nc.tensor.matmul(pdum, lhsT=wt, rhs=wt, start=True, stop=True)
            pdum2 = pspool.tile([128, 256], F32, tag="pdum", bufs=1)
            nc.tensor.matmul(pdum2, lhsT=wr2[:, :128], rhs=wr2, start=True, stop=True)

            for g in range(n_img // group):
                xg = xpool.tile([128, group, 128], F32, tag="xg")
                nc.sync.dma_start(
                    out=xg,
                    in_=x[g * group:(g + 1) * group].rearrange("i h w -> h i w"))
                og = opool.tile([128, group, 128], F32, tag="og")
                for q in range(group // 4):
                    p1 = pspool.tile([128, 4, 128], F32, tag="p1", bufs=p1_bufs)
                    for i in range(4):
                        nc.tensor.matmul(p1[:, i, :], lhsT=xg[:, q * 4 + i, :],
                                         rhs=wt, start=True, stop=True)
                    zt = zpool.tile([128, 4, 128], F32R, tag="zt")
                    nc.scalar.copy(out=zt, in_=p1)  # batched cast copy (ACT)
                    for h in range(2):
                        p2 = pspool.tile([128, 2, 256], F32, tag="p2", bufs=p2_bufs)
                        for i in range(2):
                            nc.tensor.matmul(p2[:, i, :], lhsT=zt[:, h * 2 + i, :],
                                             rhs=wr2, start=True, stop=True)
                        nc.vector.tensor_copy(
                            out=og[:, q * 4 + h * 2: q * 4 + h * 2 + 2, :],
                            in_=p2[:, :, :128])
                nc.sync.dma_start(
                    out=out[g * group:(g + 1) * group].rearrange("i h w -> h i w"),
                    in_=og)
    nc.finalize()
    return nc


_CACHE = {}


def kernel(x: np.ndarray) -> np.ndarray:
    from concourse.bass_utils import run_bass_kernel_spmd

    assert x.shape == (B, C, H, W), x.shape
    x = np.ascontiguousarray(x, dtype=np.float32)

    if "nc" not in _CACHE:
        _CACHE["nc"] = _build_program(PER_CORE)
    nc = _CACHE["nc"]

    wt = _dct_matrix(128).T.copy().astype(np.float32)       # WT[h,k] = W[k,h]
    wr2 = np.concatenate([wt, wt], axis=1).astype(np.float32)

    flat = x.reshape(B * C, H, W)
    in_maps = [
        {"x": flat[c * PER_CORE:(c + 1) * PER_CORE], "wt": wt, "wr2": wr2}
        for c in range(N_CORES)
    ]
    res = run_bass_kernel_spmd(nc, in_maps, list(range(N_CORES)))
    out = np.concatenate([r["out"] for r in res.results], axis=0)
    return out.reshape(B, C, H, W).astype(np.float32)


if __name__ == "__main__":
    rng = np.random.default_rng(0)
    xs = rng.standard_normal((B, C, H, W), dtype=np.float32)
    o = kernel(xs)
    print("kernel output", o.shape, o.dtype)

